# revision 31
# baseline (speedup 1.0000x reference)
"""Trainium2 Bass kernel for nn_Attn (dense_transformer).

Reference computation:
    proj     = einsum('sbh,oh->sbo', encoder_outputs, attn_W) + attn_b   # [S,B,H]
    energies = einsum('sbh,bh->bs', proj, hidden[0])                     # [B,S]
    out      = log_softmax(energies, axis=-1)[:, None, :]                # [B,1,S]

Algebraic rewrite:
    energies[b,s] = enc[s,b,:] . v[b]  with  v = hidden @ W  (the attn_b
    term is constant per b and cancels inside log_softmax).

Kernel strategy (TensorE-centric, latency-pipelined; ~58us vs 96.6us
baseline):
  - Data-parallel over batch: 4 b's per core on 8 cores, no collectives.
  - enc is pre-transposed on the host to [h, s] layout and quantized to
    fp8-e3m4 (halves HBM traffic; end-to-end rel-err ~1.2e-2 vs the 2e-2
    gate). W stays fp16 so v keeps near-full precision.
  - DMA issues cost ~700ns each serialized on a HWDGE queue, so they are
    split across the two HWDGE engines (Sync: W/hidT/half the output,
    ScalarE: enc in consumption order) and kept few: W as 4
    h-chunk-pair DMAs (host re-laid-out so each DMA completes two vT
    h-chunks), enc as 5 ramp pieces for the first s-block plus one DMA
    per remaining block ([512,512,512,256,256] s-blocks, each block's
    bytes one DMA so completions align with PE consumption).
  - vT[hc] = W^T @ hidTmask runs as its own 8-matmul PSUM group right
    after its W chunk lands (groups alternate between two PSUM banks;
    pool rotation enforces drain-before-reuse; PSUM accumulation groups
    are bank-scoped, so concurrent groups must not share a bank). The
    host-masked hidT ([o, (b, b')] diagonal) makes the vT output
    directly usable as block-masked lhsT columns.
  - The energies reduction runs as 160 matmuls with masked vT
    stationary, h-chunk-outer, so the first matmuls start as soon as
    vT[0] and the first 256KB enc piece arrive (~12us); steady-state
    cadence is the N=512 streaming limit (~216ns/matmul).
  - log-softmax is accumulated online per s-block: DVE block-max from
    PSUM, ScalarE fused exp(ps - m_g) straight from PSUM with
    accum_out, ScalarE drain for the final pass. The tail rescales the
    5 partial sums, takes ln, and subtracts/stores via DVE and
    ScalarE-Identity (bias AP) halves with one output DMA each.
"""

import numpy as np

S, B, H = 2048, 32, 1024
N_CORES = 8
B_LOC = B // N_CORES          # 4 batches per core
NB2 = B_LOC * B_LOC           # masked (b, b') width
NF = 4                        # s-blocks of 512 (PSUM free-dim limit)
SF = S // NF                  # 512
NHC = 8                       # h-chunks of 128
ENC_COLS = NHC * B_LOC * SF   # free dim of one f-block: 16384
HCW = B_LOC * SF              # columns per h-chunk within a block: 2048

_CACHE = {}


def _build():
    import os
    import concourse.bacc as bacc
    import concourse.mybir as mybir
    import concourse.tile as tile
    from contextlib import ExitStack

    dbg = os.environ.get("KDBG", "0") == "1"
    f32 = mybir.dt.float32
    f16 = mybir.dt.float16
    f8 = mybir.dt.float8e3
    nc = bacc.Bacc("TRN2", target_bir_lowering=False, debug=False,
                   num_devices=N_CORES)

    # enc layout, s-blocks [512,512,512,256,256]:
    #   enc_a[blk*128 + p, (hc, b, s512)] for the three 512-wide blocks,
    #   enc_b[blk*128 + p, (hc, b, s256)] for the two 256-wide tail blocks,
    #   with h = hc*128 + p.
    enc_a = nc.dram_tensor("enc_a", [3 * 128, NHC * B_LOC * 512], f8,
                           kind="ExternalInput").ap()
    enc_b = nc.dram_tensor("enc_b", [2 * 128, NHC * B_LOC * 256], f8,
                           kind="ExternalInput").ap()
    # hidT layout: [p, (oc, b, b')] = hid[b, oc*128+p] iff b == b', else 0
    hidT = nc.dram_tensor("hidT", [128, 8 * NB2], f16,
                          kind="ExternalInput").ap()
    # w layout: [pair*128 + p, (parity, oc, h')] =
    #   W[oc*128 + p, (2*pair + parity)*128 + h']
    w = nc.dram_tensor("w", [4 * 128, 2 * H], f16, kind="ExternalInput").ap()
    out = nc.dram_tensor("out", [B_LOC, S], f32, kind="ExternalOutput").ap()
    if dbg:
        acc_dbg = nc.dram_tensor("acc_dbg", [B_LOC, S], f32,
                                 kind="ExternalOutput").ap()

    with tile.TileContext(nc) as tc, ExitStack() as ctx:
        const_pool = ctx.enter_context(tc.tile_pool(name="const", bufs=1))
        w_pool = ctx.enter_context(tc.tile_pool(name="wpool", bufs=4))
        encq_pool = ctx.enter_context(tc.tile_pool(name="encq", bufs=2))
        encq2_pool = ctx.enter_context(tc.tile_pool(name="encq2", bufs=1))
        encf_pool = ctx.enter_context(tc.tile_pool(name="encf", bufs=2))
        scr_pool = ctx.enter_context(tc.tile_pool(name="scr", bufs=2))
        ps_pool = ctx.enter_context(tc.tile_pool(name="ps", bufs=2, space="PSUM"))
        psv_pool = ctx.enter_context(tc.tile_pool(name="psv", bufs=2, space="PSUM"))

        # Preload the ACT tables while ScalarE is idle. The exp set must
        # be resident when the per-f-block exps run, so warm Ln first.
        warm = const_pool.tile([1, 1], f32)
        nc.vector.memset(warm[:], 1.0)
        warm2 = const_pool.tile([1, 1], f32)
        nc.scalar.activation(warm2[:], warm[:], mybir.ActivationFunctionType.Ln)
        nc.scalar.activation(warm2[:], warm2[:], mybir.ActivationFunctionType.Exp)

        # ---- input DMAs: Sync queue for W/hidT, ScalarE queue for enc --
        hidT_sb = const_pool.tile([128, 8 * NB2], f16)
        nc.sync.dma_start(hidT_sb[:], hidT[:, :])

        w_tiles = []
        for pair in range(4):
            wt = w_pool.tile([128, 2 * H], f16, tag="wt")
            nc.sync.dma_start(wt[:], w[pair * 128:(pair + 1) * 128, :])
            w_tiles.append(wt)

        def w_lhsT(hc, oc):
            base = (hc % 2) * H + oc * 128
            return w_tiles[hc // 2][:, base:base + 128]

        # One enc DMA (or ramp piece) per s-block, all on the ScalarE
        # HWDGE queue in consumption order so completions self-pace.
        BLOCKS = [(0, 512), (512, 1024), (1024, 1536), (1536, 1792),
                  (1792, 2048)]
        NG = len(BLOCKS)
        enc_tiles = [[] for _ in range(NG)]  # [g] -> list of (tile, lo, hi)

        def enc_dma(g, lo, hi, pool, tag):
            # lo/hi are column offsets within block g's row-block.
            et = pool.tile([128, hi - lo], f8, tag=tag)
            if g < 3:
                dram = enc_a[g * 128:(g + 1) * 128, lo:hi]
            else:
                dram = enc_b[(g - 3) * 128:(g - 2) * 128, lo:hi]
            nc.scalar.dma_start(et[:], dram)
            enc_tiles[g].append((et, lo, hi))

        HB = B_LOC * 512                 # 2048 cols per h-chunk in 512-blocks
        enc_dma(0, 0, HB, encq_pool, "encq")              # blk0 hc0  (256KB)
        enc_dma(0, HB, 2 * HB, encq_pool, "encq")         # blk0 hc1  (256KB)
        enc_dma(0, 2 * HB, 4 * HB, encq2_pool, "encq2")   # blk0 hc2-3
        enc_dma(0, 4 * HB, 6 * HB, encq2_pool, "encq2b")  # blk0 hc4-5
        enc_dma(0, 6 * HB, 8 * HB, encq2_pool, "encq2c")  # blk0 hc6-7
        enc_dma(1, 0, 8 * HB, encf_pool, "encf")          # 2MB
        enc_dma(2, 0, 8 * HB, encf_pool, "encf")          # 2MB
        enc_dma(3, 0, NHC * B_LOC * 256, encf_pool, "encfb")  # 1MB
        enc_dma(4, 0, NHC * B_LOC * 256, encf_pool, "encfb")  # 1MB

        def enc_rhs(g, hc, b, width):
            col = (hc * B_LOC + b) * width
            for et, lo, hi in enc_tiles[g]:
                if lo <= col and col + width <= hi:
                    return et[:, col - lo:col - lo + width]
            raise AssertionError("enc slice spans pieces")

        # ---- vT[h, (b,b')] = sum_o W[o, h] * hidTmask[o, (b,b')] -------
        # One 8-matmul group per h-chunk, launched as its W chunk lands,
        # drained immediately to the fp8 masked lhsT tile.
        vTs = const_pool.tile([128, NHC * NB2], f8)
        for hc in range(NHC):
            pvt = psv_pool.tile([128, 512], f32, tag="vt")
            for oc in range(8):
                nc.tensor.matmul(
                    pvt[:, 0:NB2],
                    lhsT=w_lhsT(hc, oc),
                    rhs=hidT_sb[:, oc * NB2:(oc + 1) * NB2],
                    start=(oc == 0), stop=(oc == 7),
                    skip_group_check=True)
            nc.vector.tensor_copy(vTs[:, hc * NB2:(hc + 1) * NB2], pvt[:, 0:NB2])

        # ---- main loop: energies as PE matmuls + online softmax stats --
        # The last two s-blocks are 256 wide so the final serial stats
        # chain (reduce -> exp -> accum) runs on small tiles.
        acc = const_pool.tile([B_LOC, S], f32)
        mxs = const_pool.tile([B_LOC, NG], f32)
        nmxs = const_pool.tile([B_LOC, NG], f32)
        ssums = const_pool.tile([B_LOC, NG], f32)
        for g, (glo, ghi) in enumerate(BLOCKS):
            width = ghi - glo
            ps = ps_pool.tile([B_LOC, width], f32, tag=f"mm{width}")
            for hc in range(NHC):
                for b in range(B_LOC):
                    nc.tensor.matmul(
                        ps[:],
                        lhsT=vTs[:, hc * NB2 + b * B_LOC:
                                 hc * NB2 + (b + 1) * B_LOC],
                        rhs=enc_rhs(g, hc, b, width),
                        start=(hc == 0 and b == 0),
                        stop=(hc == NHC - 1 and b == B_LOC - 1),
                        skip_group_check=True)
            nc.vector.reduce_max(mxs[:, g:g + 1], ps[:],
                                 axis=mybir.AxisListType.X)
            nc.vector.tensor_scalar_mul(nmxs[:, g:g + 1], mxs[:, g:g + 1], -1.0)
            pexp = scr_pool.tile([B_LOC, width], f32, tag=f"pexp{width}")
            nc.scalar.activation(pexp[:], ps[:],
                                 mybir.ActivationFunctionType.Exp,
                                 bias=nmxs[:, g:g + 1], scale=1.0,
                                 accum_out=ssums[:, g:g + 1])
            nc.scalar.copy(acc[:, glo:ghi], ps[:])

        if dbg:
            nc.sync.dma_start(acc_dbg[:, :], acc[:])

        # ---- tail: combine the 4 online blocks -------------------------
        # S_b = sum_f ssums[b,f] * exp(mxs[b,f] - gmax[b]);
        # out = acc - gmax - ln(S)
        gmax = const_pool.tile([B_LOC, 1], f32)
        nc.vector.reduce_max(gmax[:], mxs[:], axis=mybir.AxisListType.X)
        ngmax = const_pool.tile([B_LOC, 1], f32)
        nc.vector.tensor_scalar_mul(ngmax[:], gmax[:], -1.0)
        sc = const_pool.tile([B_LOC, NG], f32)
        nc.scalar.activation(sc[:], mxs[:], mybir.ActivationFunctionType.Exp,
                             bias=ngmax[:, 0:1], scale=1.0)
        wsum = const_pool.tile([B_LOC, NG], f32)
        nc.vector.tensor_tensor(out=wsum[:], in0=ssums[:], in1=sc[:],
                                op=mybir.AluOpType.mult)
        stot = const_pool.tile([B_LOC, 1], f32)
        nc.vector.reduce_sum(stot[:], wsum[:], axis=mybir.AxisListType.X)
        lse = const_pool.tile([B_LOC, 1], f32)
        nc.scalar.activation(lse[:], stot[:], mybir.ActivationFunctionType.Ln)
        ofs = const_pool.tile([B_LOC, 1], f32)
        nc.vector.tensor_tensor(out=ofs[:], in0=gmax[:], in1=lse[:],
                                op=mybir.AluOpType.add)
        nofs = const_pool.tile([B_LOC, 1], f32)
        nc.vector.tensor_scalar_mul(nofs[:], ofs[:], -1.0)
        final = const_pool.tile([B_LOC, S], f32)
        SPLIT = 1280
        nc.vector.tensor_scalar(final[:, 0:SPLIT], acc[:, 0:SPLIT],
                                ofs[:, 0:1], None,
                                op0=mybir.AluOpType.subtract)
        nc.sync.dma_start(out[:, 0:SPLIT], final[:, 0:SPLIT])
        nc.scalar.activation(final[:, SPLIT:S], acc[:, SPLIT:S],
                             mybir.ActivationFunctionType.Identity,
                             bias=nofs[:, 0:1], scale=1.0)
        nc.scalar.dma_start(out[:, SPLIT:S], final[:, SPLIT:S])

    nc.compile()
    return nc


def _get_nc():
    if "nc" not in _CACHE:
        _CACHE["nc"] = _build()
    return _CACHE["nc"]


def kernel(hidden, encoder_outputs, attn_W, attn_b):
    import ml_dtypes
    from concourse.bass_utils import run_bass_kernel_spmd

    f8 = ml_dtypes.float8_e3m4
    hidden = np.asarray(hidden, dtype=np.float32)
    encoder_outputs = np.asarray(encoder_outputs, dtype=np.float32)
    attn_W = np.asarray(attn_W, dtype=np.float32)

    # w2[pair*128 + p, parity*H + oc*128 + h'] = W[oc*128+p, (2*pair+parity)*128+h']
    w4 = attn_W.reshape(8, 128, 8, 128)                  # [oc, p, hc, h']
    w5 = w4.transpose(2, 1, 0, 3)                        # [hc, p, oc, h']
    w6 = w5.reshape(4, 2, 128, 8, 128).transpose(0, 2, 1, 3, 4)
    w2 = np.ascontiguousarray(w6).reshape(4 * 128, 2 * H).astype(np.float16)

    in_maps = []
    for c in range(N_CORES):
        b0 = c * B_LOC
        # enc_a[blk, p, hc, b, s512] / enc_b[blk, p, hc, b, s256]
        enc_loc = encoder_outputs[:, b0:b0 + B_LOC, :]          # [S, 4, H]
        ea = enc_loc[:1536].reshape(3, 512, B_LOC, NHC, 128)
        ea = np.ascontiguousarray(ea.transpose(0, 4, 3, 2, 1))
        ea = ea.reshape(3 * 128, NHC * B_LOC * 512).astype(f8)
        eb = enc_loc[1536:].reshape(2, 256, B_LOC, NHC, 128)
        eb = np.ascontiguousarray(eb.transpose(0, 4, 3, 2, 1))
        eb = eb.reshape(2 * 128, NHC * B_LOC * 256).astype(f8)
        # hidT[p, (oc, b, b')] = hid[b, oc*128+p] iff b == b'
        hid_loc = hidden[0, b0:b0 + B_LOC, :]                   # [4, H]
        hidT3 = hid_loc.reshape(B_LOC, 8, 128).transpose(2, 1, 0)  # [p, oc, b]
        hidT = np.zeros((128, 8, B_LOC, B_LOC), dtype=np.float16)
        for b in range(B_LOC):
            hidT[:, :, b, b] = hidT3[:, :, b]
        hidT = hidT.reshape(128, 8 * NB2)
        in_maps.append({"enc_a": ea, "enc_b": eb, "hidT": hidT, "w": w2})

    nc = _get_nc()
    res = run_bass_kernel_spmd(nc, in_maps, core_ids=list(range(N_CORES)))
    _CACHE["last_results"] = res
    outs = [r["out"] for r in res.results]          # each [B_LOC, S]
    full = np.concatenate(outs, axis=0)             # [B, S]
    return full[:, None, :].astype(np.float32)      # [B, 1, S]


# revision 32
# speedup vs baseline: 1.0260x; 1.0260x over previous
"""Trainium2 Bass kernel for nn_Attn (dense_transformer).

Reference computation:
    proj     = einsum('sbh,oh->sbo', encoder_outputs, attn_W) + attn_b   # [S,B,H]
    energies = einsum('sbh,bh->bs', proj, hidden[0])                     # [B,S]
    out      = log_softmax(energies, axis=-1)[:, None, :]                # [B,1,S]

Algebraic rewrite:
    energies[b,s] = enc[s,b,:] . v[b]  with  v = hidden @ W  (the attn_b
    term is constant per b and cancels inside log_softmax).

Kernel strategy (TensorE-centric, latency-pipelined; ~58us vs 96.6us
baseline):
  - Data-parallel over batch: 4 b's per core on 8 cores, no collectives.
  - enc is pre-transposed on the host to [h, s] layout and quantized to
    fp8-e3m4 (halves HBM traffic; end-to-end rel-err ~1.2e-2 vs the 2e-2
    gate). W stays fp16 so v keeps near-full precision.
  - DMA issues cost ~700ns each serialized on a HWDGE queue, so they are
    split across the two HWDGE engines (Sync: W/hidT/half the output,
    ScalarE: enc in consumption order) and kept few: W as 4
    h-chunk-pair DMAs (host re-laid-out so each DMA completes two vT
    h-chunks), enc as 5 ramp pieces for the first s-block plus one DMA
    per remaining block ([512,512,512,256,256] s-blocks, each block's
    bytes one DMA so completions align with PE consumption).
  - vT[hc] = W^T @ hidTmask runs as its own 8-matmul PSUM group right
    after its W chunk lands (groups alternate between two PSUM banks;
    pool rotation enforces drain-before-reuse; PSUM accumulation groups
    are bank-scoped, so concurrent groups must not share a bank). The
    host-masked hidT ([o, (b, b')] diagonal) makes the vT output
    directly usable as block-masked lhsT columns.
  - The energies reduction runs as 160 matmuls with masked vT
    stationary, h-chunk-outer, so the first matmuls start as soon as
    vT[0] and the first 256KB enc piece arrive (~12us); steady-state
    cadence is the N=512 streaming limit (~216ns/matmul).
  - log-softmax is accumulated online per s-block: DVE block-max from
    PSUM, ScalarE fused exp(ps - m_g) straight from PSUM with
    accum_out, ScalarE drain for the final pass. The tail rescales the
    5 partial sums, takes ln, and subtracts/stores via DVE and
    ScalarE-Identity (bias AP) halves with one output DMA each.
"""

import numpy as np

S, B, H = 2048, 32, 1024
N_CORES = 8
B_LOC = B // N_CORES          # 4 batches per core
NB2 = B_LOC * B_LOC           # masked (b, b') width
NF = 4                        # s-blocks of 512 (PSUM free-dim limit)
SF = S // NF                  # 512
NHC = 8                       # h-chunks of 128
ENC_COLS = NHC * B_LOC * SF   # free dim of one f-block: 16384
HCW = B_LOC * SF              # columns per h-chunk within a block: 2048

_CACHE = {}


def _build():
    import os
    import concourse.bacc as bacc
    import concourse.mybir as mybir
    import concourse.tile as tile
    from contextlib import ExitStack

    dbg = os.environ.get("KDBG", "0") == "1"
    f32 = mybir.dt.float32
    f16 = mybir.dt.float16
    f8 = mybir.dt.float8e3
    nc = bacc.Bacc("TRN2", target_bir_lowering=False, debug=False,
                   num_devices=N_CORES)

    # enc layout, s-blocks [512,512,512,256,256]:
    #   enc_a[blk*128 + p, (hc, b, s512)] for the three 512-wide blocks,
    #   enc_b[blk*128 + p, (hc, b, s256)] for the two 256-wide tail blocks,
    #   with h = hc*128 + p.
    enc_a = nc.dram_tensor("enc_a", [3 * 128, NHC * B_LOC * 512], f8,
                           kind="ExternalInput").ap()
    enc_b = nc.dram_tensor("enc_b", [2 * 128, NHC * B_LOC * 256], f8,
                           kind="ExternalInput").ap()
    # hidT layout: [p, (oc, b, b')] = hid[b, oc*128+p] iff b == b', else 0
    hidT = nc.dram_tensor("hidT", [128, 8 * NB2], f16,
                          kind="ExternalInput").ap()
    # w layout: [pair*128 + p, (parity, oc, h')] =
    #   W[oc*128 + p, (2*pair + parity)*128 + h']
    w = nc.dram_tensor("w", [4 * 128, 2 * H], f16, kind="ExternalInput").ap()
    out = nc.dram_tensor("out", [B_LOC, S], f32, kind="ExternalOutput").ap()
    if dbg:
        acc_dbg = nc.dram_tensor("acc_dbg", [B_LOC, S], f32,
                                 kind="ExternalOutput").ap()

    with tile.TileContext(nc) as tc, ExitStack() as ctx:
        const_pool = ctx.enter_context(tc.tile_pool(name="const", bufs=1))
        w_pool = ctx.enter_context(tc.tile_pool(name="wpool", bufs=4))
        encq_pool = ctx.enter_context(tc.tile_pool(name="encq", bufs=2))
        encq2_pool = ctx.enter_context(tc.tile_pool(name="encq2", bufs=1))
        encf_pool = ctx.enter_context(tc.tile_pool(name="encf", bufs=2))
        scr_pool = ctx.enter_context(tc.tile_pool(name="scr", bufs=2))
        ps_pool = ctx.enter_context(tc.tile_pool(name="ps", bufs=2, space="PSUM"))
        psv_pool = ctx.enter_context(tc.tile_pool(name="psv", bufs=2, space="PSUM"))

        # Preload the ACT tables while ScalarE is idle. The exp set must
        # be resident when the per-f-block exps run, so warm Ln first.
        warm = const_pool.tile([1, 1], f32)
        nc.vector.memset(warm[:], 1.0)
        warm2 = const_pool.tile([1, 1], f32)
        nc.scalar.activation(warm2[:], warm[:], mybir.ActivationFunctionType.Ln)
        nc.scalar.activation(warm2[:], warm2[:], mybir.ActivationFunctionType.Exp)

        # ---- input DMAs: Sync queue for W/hidT, ScalarE queue for enc --
        hidT_sb = const_pool.tile([128, 8 * NB2], f16)
        nc.sync.dma_start(hidT_sb[:], hidT[:, :])

        w_tiles = []
        for pair in range(4):
            wt = w_pool.tile([128, 2 * H], f16, tag="wt")
            nc.sync.dma_start(wt[:], w[pair * 128:(pair + 1) * 128, :])
            w_tiles.append(wt)

        def w_lhsT(hc, oc):
            base = (hc % 2) * H + oc * 128
            return w_tiles[hc // 2][:, base:base + 128]

        # One enc DMA (or ramp piece) per s-block, all on the ScalarE
        # HWDGE queue in consumption order so completions self-pace.
        BLOCKS = [(0, 512), (512, 1024), (1024, 1536), (1536, 1792),
                  (1792, 2048)]
        NG = len(BLOCKS)
        enc_tiles = [[] for _ in range(NG)]  # [g] -> list of (tile, lo, hi)

        def enc_dma(g, lo, hi, pool, tag):
            # lo/hi are column offsets within block g's row-block.
            et = pool.tile([128, hi - lo], f8, tag=tag)
            if g < 3:
                dram = enc_a[g * 128:(g + 1) * 128, lo:hi]
            else:
                dram = enc_b[(g - 3) * 128:(g - 2) * 128, lo:hi]
            nc.scalar.dma_start(et[:], dram)
            enc_tiles[g].append((et, lo, hi))

        HB = B_LOC * 512                 # 2048 cols per h-chunk in 512-blocks
        enc_dma(0, 0, HB, encq_pool, "encq")              # blk0 hc0  (256KB)
        enc_dma(0, HB, 2 * HB, encq_pool, "encq")         # blk0 hc1  (256KB)
        enc_dma(0, 2 * HB, 4 * HB, encq2_pool, "encq2")   # blk0 hc2-3
        enc_dma(0, 4 * HB, 6 * HB, encq2_pool, "encq2b")  # blk0 hc4-5
        enc_dma(0, 6 * HB, 8 * HB, encq2_pool, "encq2c")  # blk0 hc6-7
        enc_dma(1, 0, 8 * HB, encf_pool, "encf")          # 2MB
        enc_dma(2, 0, 8 * HB, encf_pool, "encf")          # 2MB
        enc_dma(3, 0, NHC * B_LOC * 256, encf_pool, "encfb")  # 1MB
        enc_dma(4, 0, NHC * B_LOC * 256, encf_pool, "encfb")  # 1MB

        def enc_rhs(g, hc, b, width):
            col = (hc * B_LOC + b) * width
            for et, lo, hi in enc_tiles[g]:
                if lo <= col and col + width <= hi:
                    return et[:, col - lo:col - lo + width]
            raise AssertionError("enc slice spans pieces")

        # ---- vT[h, (b,b')] = sum_o W[o, h] * hidTmask[o, (b,b')] -------
        # One 8-matmul group per h-chunk, launched as its W chunk lands,
        # drained immediately to the fp8 masked lhsT tile.
        vTs = const_pool.tile([128, NHC * NB2], f16)
        for hc in range(NHC):
            pvt = psv_pool.tile([128, 512], f32, tag="vt")
            for oc in range(8):
                nc.tensor.matmul(
                    pvt[:, 0:NB2],
                    lhsT=w_lhsT(hc, oc),
                    rhs=hidT_sb[:, oc * NB2:(oc + 1) * NB2],
                    start=(oc == 0), stop=(oc == 7),
                    skip_group_check=True)
            nc.vector.tensor_copy(vTs[:, hc * NB2:(hc + 1) * NB2], pvt[:, 0:NB2])

        # ---- main loop: energies as PE matmuls + online softmax stats --
        # The last two s-blocks are 256 wide so the final serial stats
        # chain (reduce -> exp -> accum) runs on small tiles.
        acc = const_pool.tile([B_LOC, S], f32)
        mxs = const_pool.tile([B_LOC, NG], f32)
        nmxs = const_pool.tile([B_LOC, NG], f32)
        ssums = const_pool.tile([B_LOC, NG], f32)
        for g, (glo, ghi) in enumerate(BLOCKS):
            width = ghi - glo
            ps = ps_pool.tile([B_LOC, width], f32, tag=f"mm{width}")
            for hc in range(NHC):
                for b in range(B_LOC):
                    nc.tensor.matmul(
                        ps[:],
                        lhsT=vTs[:, hc * NB2 + b * B_LOC:
                                 hc * NB2 + (b + 1) * B_LOC],
                        rhs=enc_rhs(g, hc, b, width),
                        start=(hc == 0 and b == 0),
                        stop=(hc == NHC - 1 and b == B_LOC - 1),
                        skip_group_check=True)
            nc.vector.reduce_max(mxs[:, g:g + 1], ps[:],
                                 axis=mybir.AxisListType.X)
            nc.vector.tensor_scalar_mul(nmxs[:, g:g + 1], mxs[:, g:g + 1], -1.0)
            pexp = scr_pool.tile([B_LOC, width], f32, tag=f"pexp{width}")
            nc.scalar.activation(pexp[:], ps[:],
                                 mybir.ActivationFunctionType.Exp,
                                 bias=nmxs[:, g:g + 1], scale=1.0,
                                 accum_out=ssums[:, g:g + 1])
            nc.scalar.copy(acc[:, glo:ghi], ps[:])

        if dbg:
            nc.sync.dma_start(acc_dbg[:, :], acc[:])

        # ---- tail: combine the 4 online blocks -------------------------
        # S_b = sum_f ssums[b,f] * exp(mxs[b,f] - gmax[b]);
        # out = acc - gmax - ln(S)
        gmax = const_pool.tile([B_LOC, 1], f32)
        nc.vector.reduce_max(gmax[:], mxs[:], axis=mybir.AxisListType.X)
        ngmax = const_pool.tile([B_LOC, 1], f32)
        nc.vector.tensor_scalar_mul(ngmax[:], gmax[:], -1.0)
        sc = const_pool.tile([B_LOC, NG], f32)
        nc.scalar.activation(sc[:], mxs[:], mybir.ActivationFunctionType.Exp,
                             bias=ngmax[:, 0:1], scale=1.0)
        wsum = const_pool.tile([B_LOC, NG], f32)
        nc.vector.tensor_tensor(out=wsum[:], in0=ssums[:], in1=sc[:],
                                op=mybir.AluOpType.mult)
        stot = const_pool.tile([B_LOC, 1], f32)
        nc.vector.reduce_sum(stot[:], wsum[:], axis=mybir.AxisListType.X)
        lse = const_pool.tile([B_LOC, 1], f32)
        nc.scalar.activation(lse[:], stot[:], mybir.ActivationFunctionType.Ln)
        ofs = const_pool.tile([B_LOC, 1], f32)
        nc.vector.tensor_tensor(out=ofs[:], in0=gmax[:], in1=lse[:],
                                op=mybir.AluOpType.add)
        nofs = const_pool.tile([B_LOC, 1], f32)
        nc.vector.tensor_scalar_mul(nofs[:], ofs[:], -1.0)
        final = const_pool.tile([B_LOC, S], f32)
        SPLIT = 1280
        nc.vector.tensor_scalar(final[:, 0:SPLIT], acc[:, 0:SPLIT],
                                ofs[:, 0:1], None,
                                op0=mybir.AluOpType.subtract)
        nc.sync.dma_start(out[:, 0:SPLIT], final[:, 0:SPLIT])
        nc.scalar.activation(final[:, SPLIT:S], acc[:, SPLIT:S],
                             mybir.ActivationFunctionType.Identity,
                             bias=nofs[:, 0:1], scale=1.0)
        nc.scalar.dma_start(out[:, SPLIT:S], final[:, SPLIT:S])

    nc.compile()
    return nc


def _get_nc():
    if "nc" not in _CACHE:
        _CACHE["nc"] = _build()
    return _CACHE["nc"]


def kernel(hidden, encoder_outputs, attn_W, attn_b):
    import ml_dtypes
    from concourse.bass_utils import run_bass_kernel_spmd

    f8 = ml_dtypes.float8_e3m4
    hidden = np.asarray(hidden, dtype=np.float32)
    encoder_outputs = np.asarray(encoder_outputs, dtype=np.float32)
    attn_W = np.asarray(attn_W, dtype=np.float32)

    # w2[pair*128 + p, parity*H + oc*128 + h'] = W[oc*128+p, (2*pair+parity)*128+h']
    w4 = attn_W.reshape(8, 128, 8, 128)                  # [oc, p, hc, h']
    w5 = w4.transpose(2, 1, 0, 3)                        # [hc, p, oc, h']
    w6 = w5.reshape(4, 2, 128, 8, 128).transpose(0, 2, 1, 3, 4)
    w2 = np.ascontiguousarray(w6).reshape(4 * 128, 2 * H).astype(np.float16)

    in_maps = []
    for c in range(N_CORES):
        b0 = c * B_LOC
        # enc_a[blk, p, hc, b, s512] / enc_b[blk, p, hc, b, s256]
        enc_loc = encoder_outputs[:, b0:b0 + B_LOC, :]          # [S, 4, H]
        ea = enc_loc[:1536].reshape(3, 512, B_LOC, NHC, 128)
        ea = np.ascontiguousarray(ea.transpose(0, 4, 3, 2, 1))
        ea = ea.reshape(3 * 128, NHC * B_LOC * 512).astype(f8)
        eb = enc_loc[1536:].reshape(2, 256, B_LOC, NHC, 128)
        eb = np.ascontiguousarray(eb.transpose(0, 4, 3, 2, 1))
        eb = eb.reshape(2 * 128, NHC * B_LOC * 256).astype(f8)
        # hidT[p, (oc, b, b')] = hid[b, oc*128+p] iff b == b'
        hid_loc = hidden[0, b0:b0 + B_LOC, :]                   # [4, H]
        hidT3 = hid_loc.reshape(B_LOC, 8, 128).transpose(2, 1, 0)  # [p, oc, b]
        hidT = np.zeros((128, 8, B_LOC, B_LOC), dtype=np.float16)
        for b in range(B_LOC):
            hidT[:, :, b, b] = hidT3[:, :, b]
        hidT = hidT.reshape(128, 8 * NB2)
        in_maps.append({"enc_a": ea, "enc_b": eb, "hidT": hidT, "w": w2})

    nc = _get_nc()
    res = run_bass_kernel_spmd(nc, in_maps, core_ids=list(range(N_CORES)))
    _CACHE["last_results"] = res
    outs = [r["out"] for r in res.results]          # each [B_LOC, S]
    full = np.concatenate(outs, axis=0)             # [B, S]
    return full[:, None, :].astype(np.float32)      # [B, 1, S]


# revision 33
# speedup vs baseline: 1.0642x; 1.0372x over previous
"""Trainium2 Bass kernel for nn_Attn (dense_transformer).

Reference computation:
    proj     = einsum('sbh,oh->sbo', encoder_outputs, attn_W) + attn_b   # [S,B,H]
    energies = einsum('sbh,bh->bs', proj, hidden[0])                     # [B,S]
    out      = log_softmax(energies, axis=-1)[:, None, :]                # [B,1,S]

Algebraic rewrite:
    energies[b,s] = enc[s,b,:] . v[b]  with  v = hidden @ W  (the attn_b
    term is constant per b and cancels inside log_softmax).

Kernel strategy (TensorE-centric, latency-pipelined; ~58us vs 96.6us
baseline):
  - Data-parallel over batch: 4 b's per core on 8 cores, no collectives.
  - enc is pre-transposed on the host to [h, s] layout and quantized to
    fp8-e3m4 (halves HBM traffic; end-to-end rel-err ~1.2e-2 vs the 2e-2
    gate). W stays fp16 so v keeps near-full precision.
  - DMA issues cost ~700ns each serialized on a HWDGE queue, so they are
    split across the two HWDGE engines (Sync: W/hidT/half the output,
    ScalarE: enc in consumption order) and kept few: W as 4
    h-chunk-pair DMAs (host re-laid-out so each DMA completes two vT
    h-chunks), enc as 5 ramp pieces for the first s-block plus one DMA
    per remaining block ([512,512,512,256,256] s-blocks, each block's
    bytes one DMA so completions align with PE consumption).
  - vT[hc] = W^T @ hidTmask runs as its own 8-matmul PSUM group right
    after its W chunk lands (groups alternate between two PSUM banks;
    pool rotation enforces drain-before-reuse; PSUM accumulation groups
    are bank-scoped, so concurrent groups must not share a bank). The
    host-masked hidT ([o, (b, b')] diagonal) makes the vT output
    directly usable as block-masked lhsT columns.
  - The energies reduction runs as 160 matmuls with masked vT
    stationary, h-chunk-outer, so the first matmuls start as soon as
    vT[0] and the first 256KB enc piece arrive (~12us); steady-state
    cadence is the N=512 streaming limit (~216ns/matmul).
  - log-softmax is accumulated online per s-block: DVE block-max from
    PSUM, ScalarE fused exp(ps - m_g) straight from PSUM with
    accum_out, ScalarE drain for the final pass. The tail rescales the
    5 partial sums, takes ln, and subtracts/stores via DVE and
    ScalarE-Identity (bias AP) halves with one output DMA each.
"""

import numpy as np

S, B, H = 2048, 32, 1024
N_CORES = 8
B_LOC = B // N_CORES          # 4 batches per core
NB2 = B_LOC * B_LOC           # masked (b, b') width
NF = 4                        # s-blocks of 512 (PSUM free-dim limit)
SF = S // NF                  # 512
NHC = 8                       # h-chunks of 128
ENC_COLS = NHC * B_LOC * SF   # free dim of one f-block: 16384
HCW = B_LOC * SF              # columns per h-chunk within a block: 2048

_CACHE = {}


def _build():
    import os
    import concourse.bacc as bacc
    import concourse.mybir as mybir
    import concourse.tile as tile
    from contextlib import ExitStack

    dbg = os.environ.get("KDBG", "0") == "1"
    f32 = mybir.dt.float32
    f16 = mybir.dt.float16
    f8 = mybir.dt.float8e3
    nc = bacc.Bacc("TRN2", target_bir_lowering=False, debug=False,
                   num_devices=N_CORES)

    # enc layout, s-blocks [512,512,512,256,256]:
    #   enc_a[blk*128 + p, (hc, b, s512)] for the three 512-wide blocks,
    #   enc_b[blk*128 + p, (hc, b, s256)] for the two 256-wide tail blocks,
    #   with h = hc*128 + p.
    enc_a = nc.dram_tensor("enc_a", [3 * 128, NHC * B_LOC * 512], f8,
                           kind="ExternalInput").ap()
    enc_b = nc.dram_tensor("enc_b", [2 * 128, NHC * B_LOC * 256], f8,
                           kind="ExternalInput").ap()
    # hidT layout: [p, (oc, b, b')] = hid[b, oc*128+p] iff b == b', else 0
    hidT = nc.dram_tensor("hidT", [128, 8 * NB2], f16,
                          kind="ExternalInput").ap()
    # w layout: [pair*128 + p, (parity, oc, h')] =
    #   W[oc*128 + p, (2*pair + parity)*128 + h']
    w = nc.dram_tensor("w", [4 * 128, 2 * H], f8, kind="ExternalInput").ap()
    out = nc.dram_tensor("out", [B_LOC, S], f32, kind="ExternalOutput").ap()
    if dbg:
        acc_dbg = nc.dram_tensor("acc_dbg", [B_LOC, S], f32,
                                 kind="ExternalOutput").ap()

    with tile.TileContext(nc) as tc, ExitStack() as ctx:
        const_pool = ctx.enter_context(tc.tile_pool(name="const", bufs=1))
        w_pool = ctx.enter_context(tc.tile_pool(name="wpool", bufs=4))
        encq_pool = ctx.enter_context(tc.tile_pool(name="encq", bufs=2))
        encq2_pool = ctx.enter_context(tc.tile_pool(name="encq2", bufs=1))
        encf_pool = ctx.enter_context(tc.tile_pool(name="encf", bufs=2))
        scr_pool = ctx.enter_context(tc.tile_pool(name="scr", bufs=2))
        ps_pool = ctx.enter_context(tc.tile_pool(name="ps", bufs=2, space="PSUM"))
        psv_pool = ctx.enter_context(tc.tile_pool(name="psv", bufs=2, space="PSUM"))

        # Preload the ACT tables while ScalarE is idle. The exp set must
        # be resident when the per-f-block exps run, so warm Ln first.
        warm = const_pool.tile([1, 1], f32)
        nc.vector.memset(warm[:], 1.0)
        warm2 = const_pool.tile([1, 1], f32)
        nc.scalar.activation(warm2[:], warm[:], mybir.ActivationFunctionType.Ln)
        nc.scalar.activation(warm2[:], warm2[:], mybir.ActivationFunctionType.Exp)

        # ---- input DMAs: Sync queue for W/hidT, ScalarE queue for enc --
        hidT_sb = const_pool.tile([128, 8 * NB2], f16)
        nc.sync.dma_start(hidT_sb[:], hidT[:, :])

        w_tiles = []
        for pair in range(4):
            wt = w_pool.tile([128, 2 * H], f8, tag="wt")
            nc.sync.dma_start(wt[:], w[pair * 128:(pair + 1) * 128, :])
            w_tiles.append(wt)

        def w_lhsT(hc, oc):
            base = (hc % 2) * H + oc * 128
            return w_tiles[hc // 2][:, base:base + 128]

        # One enc DMA (or ramp piece) per s-block, all on the ScalarE
        # HWDGE queue in consumption order so completions self-pace.
        BLOCKS = [(0, 512), (512, 1024), (1024, 1536), (1536, 1792),
                  (1792, 2048)]
        NG = len(BLOCKS)
        enc_tiles = [[] for _ in range(NG)]  # [g] -> list of (tile, lo, hi)

        def enc_dma(g, lo, hi, pool, tag):
            # lo/hi are column offsets within block g's row-block.
            et = pool.tile([128, hi - lo], f8, tag=tag)
            if g < 3:
                dram = enc_a[g * 128:(g + 1) * 128, lo:hi]
            else:
                dram = enc_b[(g - 3) * 128:(g - 2) * 128, lo:hi]
            nc.scalar.dma_start(et[:], dram)
            enc_tiles[g].append((et, lo, hi))

        HB = B_LOC * 512                 # 2048 cols per h-chunk in 512-blocks
        enc_dma(0, 0, HB, encq_pool, "encq")              # blk0 hc0  (256KB)
        enc_dma(0, HB, 2 * HB, encq_pool, "encq")         # blk0 hc1  (256KB)
        enc_dma(0, 2 * HB, 4 * HB, encq2_pool, "encq2")   # blk0 hc2-3
        enc_dma(0, 4 * HB, 6 * HB, encq2_pool, "encq2b")  # blk0 hc4-5
        enc_dma(0, 6 * HB, 8 * HB, encq2_pool, "encq2c")  # blk0 hc6-7
        enc_dma(1, 0, 8 * HB, encf_pool, "encf")          # 2MB
        enc_dma(2, 0, 8 * HB, encf_pool, "encf")          # 2MB
        enc_dma(3, 0, NHC * B_LOC * 256, encf_pool, "encfb")  # 1MB
        enc_dma(4, 0, NHC * B_LOC * 256, encf_pool, "encfb")  # 1MB

        def enc_rhs(g, hc, b, width):
            col = (hc * B_LOC + b) * width
            for et, lo, hi in enc_tiles[g]:
                if lo <= col and col + width <= hi:
                    return et[:, col - lo:col - lo + width]
            raise AssertionError("enc slice spans pieces")

        # ---- vT[h, (b,b')] = sum_o W[o, h] * hidTmask[o, (b,b')] -------
        # One 8-matmul group per h-chunk, launched as its W chunk lands,
        # drained immediately to the fp8 masked lhsT tile.
        vTs = const_pool.tile([128, NHC * NB2], f16)
        for hc in range(NHC):
            pvt = psv_pool.tile([128, 512], f32, tag="vt")
            for oc in range(8):
                nc.tensor.matmul(
                    pvt[:, 0:NB2],
                    lhsT=w_lhsT(hc, oc),
                    rhs=hidT_sb[:, oc * NB2:(oc + 1) * NB2],
                    start=(oc == 0), stop=(oc == 7),
                    skip_group_check=True)
            nc.vector.tensor_copy(vTs[:, hc * NB2:(hc + 1) * NB2], pvt[:, 0:NB2])

        # ---- main loop: energies as PE matmuls + online softmax stats --
        # The last two s-blocks are 256 wide so the final serial stats
        # chain (reduce -> exp -> accum) runs on small tiles.
        acc = const_pool.tile([B_LOC, S], f32)
        mxs = const_pool.tile([B_LOC, NG], f32)
        nmxs = const_pool.tile([B_LOC, NG], f32)
        ssums = const_pool.tile([B_LOC, NG], f32)
        for g, (glo, ghi) in enumerate(BLOCKS):
            width = ghi - glo
            ps = ps_pool.tile([B_LOC, width], f32, tag=f"mm{width}")
            for hc in range(NHC):
                for b in range(B_LOC):
                    nc.tensor.matmul(
                        ps[:],
                        lhsT=vTs[:, hc * NB2 + b * B_LOC:
                                 hc * NB2 + (b + 1) * B_LOC],
                        rhs=enc_rhs(g, hc, b, width),
                        start=(hc == 0 and b == 0),
                        stop=(hc == NHC - 1 and b == B_LOC - 1),
                        skip_group_check=True)
            nc.vector.reduce_max(mxs[:, g:g + 1], ps[:],
                                 axis=mybir.AxisListType.X)
            nc.vector.tensor_scalar_mul(nmxs[:, g:g + 1], mxs[:, g:g + 1],
                                        -1.0 / 32.0)
            pexp = scr_pool.tile([B_LOC, width], f32, tag=f"pexp{width}")
            nc.scalar.activation(pexp[:], ps[:],
                                 mybir.ActivationFunctionType.Exp,
                                 bias=nmxs[:, g:g + 1], scale=1.0 / 32.0,
                                 accum_out=ssums[:, g:g + 1])
            nc.scalar.activation(acc[:, glo:ghi], ps[:],
                                 mybir.ActivationFunctionType.Copy,
                                 bias=0.0, scale=1.0 / 32.0)

        if dbg:
            nc.sync.dma_start(acc_dbg[:, :], acc[:])

        # ---- tail: combine the 4 online blocks -------------------------
        # S_b = sum_f ssums[b,f] * exp(mxs[b,f] - gmax[b]);
        # out = acc - gmax - ln(S)
        gmax = const_pool.tile([B_LOC, 1], f32)
        nc.vector.reduce_max(gmax[:], mxs[:], axis=mybir.AxisListType.X)
        ngmax = const_pool.tile([B_LOC, 1], f32)
        nc.vector.tensor_scalar_mul(ngmax[:], gmax[:], -1.0 / 32.0)
        sc = const_pool.tile([B_LOC, NG], f32)
        nc.scalar.activation(sc[:], mxs[:], mybir.ActivationFunctionType.Exp,
                             bias=ngmax[:, 0:1], scale=1.0 / 32.0)
        wsum = const_pool.tile([B_LOC, NG], f32)
        nc.vector.tensor_tensor(out=wsum[:], in0=ssums[:], in1=sc[:],
                                op=mybir.AluOpType.mult)
        stot = const_pool.tile([B_LOC, 1], f32)
        nc.vector.reduce_sum(stot[:], wsum[:], axis=mybir.AxisListType.X)
        lse = const_pool.tile([B_LOC, 1], f32)
        nc.scalar.activation(lse[:], stot[:], mybir.ActivationFunctionType.Ln)
        ofs = const_pool.tile([B_LOC, 1], f32)
        nc.vector.tensor_tensor(out=ofs[:], in0=lse[:], in1=ngmax[:],
                                op=mybir.AluOpType.subtract)
        nofs = const_pool.tile([B_LOC, 1], f32)
        nc.vector.tensor_scalar_mul(nofs[:], ofs[:], -1.0)
        final = const_pool.tile([B_LOC, S], f32)
        SPLIT = 1280
        nc.vector.tensor_scalar(final[:, 0:SPLIT], acc[:, 0:SPLIT],
                                ofs[:, 0:1], None,
                                op0=mybir.AluOpType.subtract)
        nc.sync.dma_start(out[:, 0:SPLIT], final[:, 0:SPLIT])
        nc.scalar.activation(final[:, SPLIT:S], acc[:, SPLIT:S],
                             mybir.ActivationFunctionType.Identity,
                             bias=nofs[:, 0:1], scale=1.0)
        nc.scalar.dma_start(out[:, SPLIT:S], final[:, SPLIT:S])

    nc.compile()
    return nc


def _get_nc():
    if "nc" not in _CACHE:
        _CACHE["nc"] = _build()
    return _CACHE["nc"]


def kernel(hidden, encoder_outputs, attn_W, attn_b):
    import ml_dtypes
    from concourse.bass_utils import run_bass_kernel_spmd

    f8 = ml_dtypes.float8_e3m4
    hidden = np.asarray(hidden, dtype=np.float32)
    encoder_outputs = np.asarray(encoder_outputs, dtype=np.float32)
    attn_W = np.asarray(attn_W, dtype=np.float32)

    # w2[pair*128 + p, parity*H + oc*128 + h'] = W[oc*128+p, (2*pair+parity)*128+h']
    w4 = attn_W.reshape(8, 128, 8, 128)                  # [oc, p, hc, h']
    w5 = w4.transpose(2, 1, 0, 3)                        # [hc, p, oc, h']
    w6 = w5.reshape(4, 2, 128, 8, 128).transpose(0, 2, 1, 3, 4)
    w2 = (np.ascontiguousarray(w6).reshape(4 * 128, 2 * H) * 32.0).astype(f8)

    in_maps = []
    for c in range(N_CORES):
        b0 = c * B_LOC
        # enc_a[blk, p, hc, b, s512] / enc_b[blk, p, hc, b, s256]
        enc_loc = encoder_outputs[:, b0:b0 + B_LOC, :]          # [S, 4, H]
        ea = enc_loc[:1536].reshape(3, 512, B_LOC, NHC, 128)
        ea = np.ascontiguousarray(ea.transpose(0, 4, 3, 2, 1))
        ea = ea.reshape(3 * 128, NHC * B_LOC * 512).astype(f8)
        eb = enc_loc[1536:].reshape(2, 256, B_LOC, NHC, 128)
        eb = np.ascontiguousarray(eb.transpose(0, 4, 3, 2, 1))
        eb = eb.reshape(2 * 128, NHC * B_LOC * 256).astype(f8)
        # hidT[p, (oc, b, b')] = hid[b, oc*128+p] iff b == b'
        hid_loc = hidden[0, b0:b0 + B_LOC, :]                   # [4, H]
        hidT3 = hid_loc.reshape(B_LOC, 8, 128).transpose(2, 1, 0)  # [p, oc, b]
        hidT = np.zeros((128, 8, B_LOC, B_LOC), dtype=np.float16)
        for b in range(B_LOC):
            hidT[:, :, b, b] = hidT3[:, :, b]
        hidT = hidT.reshape(128, 8 * NB2)
        in_maps.append({"enc_a": ea, "enc_b": eb, "hidT": hidT, "w": w2})

    nc = _get_nc()
    res = run_bass_kernel_spmd(nc, in_maps, core_ids=list(range(N_CORES)))
    _CACHE["last_results"] = res
    outs = [r["out"] for r in res.results]          # each [B_LOC, S]
    full = np.concatenate(outs, axis=0)             # [B, S]
    return full[:, None, :].astype(np.float32)      # [B, 1, S]
